# revision 60
# baseline (speedup 1.0000x reference)
"""Trainium2 Bass kernel for nn_DressedQuantumNet (262144 x 64 -> 262144 x 1).

Design G: host-pretransposed input (pure linear DMAs) + double-angle
quadratic form.

Math: with t = tanh(u), u = x @ pre_w.T + pre_b, the circuit output is
    y = h^T M16 h,   h = (C0,S0,C1,S1,C2,S2,C3,S3, C0C1,C0S1,S0C1,S0S1,
                          C2C3,C2S3,S2C3,S2S3)
where Cw = cos 2phi_w = -sin((pi/2) t_w), Sw = sin 2phi_w = cos((pi/2) t_w),
and M16 (16x16 symmetric, absorbing post_w/post_b via c^2+s^2=1 identities)
is solved on host by least squares.

Per-core layout (S = 32768 samples, 4 macros of 8192):
  sample s = 8192 m + 64 p + 32 ut + 8 v + 2 c + j   (p<128, ut<2, v<4, c<4, j<2)
  xt dram [m][64j+f, 512(4ut+v) + 128c + p] = x[s, f]        (host-baked)
  pre-matmul K=128 (2 samples j), M=32 blocks (4v): U[32v+4j+i, 128c+p]
  tanh -> T f16 [128, 1024] per m;  xbar -> T2s[p, 128(4ut+c) + 32v+4j+i]
  ACT sin/cos -> H[p, 1024m + 128(4ut+c) + 8st + (2v+j)]  raw slots st=2w+ph
  DVE products -> slots 8..15;  xbar H -> HT[8st+q, 128(4ut+c)+p]
  MP = m16bd^T HT (q-interleaved blockdiag), DP = HT*MP,
  RSUM k4=2(m%2)+ut accumulates into Yp rows 8 k4 + q -> y[P, 32, 512] f32.
Host un-permutes the output.
"""
import sys

import numpy as np

for _p in ("/opt/trn_rl_repo",):
    if _p not in sys.path:
        sys.path.insert(0, _p)

import concourse.bass as bass
import concourse.bacc as bacc
import concourse.hw_specs as _hw_specs

_orig_get_act_tables = _hw_specs.get_activation_tables


def _pinned_act_tables(module_arch):
    tabs = _orig_get_act_tables(module_arch)
    if "silu_and_others" in tabs:
        tabs = {k: (v if k == "silu_and_others" else set())
                for k, v in tabs.items()}
    return tabs


bacc.get_activation_tables = _pinned_act_tables
import concourse.mybir as mybir
from concourse import tile
from concourse.bass_utils import run_bass_kernel_spmd

AF = mybir.ActivationFunctionType
ALU = mybir.AluOpType
F32 = mybir.dt.float32
F16 = mybir.dt.float16

N_CORES = 8
BATCH = 262144
S = BATCH // N_CORES          # 32768 samples per core
NM = 4                        # macros per core (8192 samples each)
N_QUBITS = 4
Q_DEPTH = 6
IN_F = 64

TRACE = False
LAST_RESULTS = None

# ---------------------------------------------------------------- host math


def _ry(theta):
    c, s = np.cos(theta / 2), np.sin(theta / 2)
    return np.array([[c, -s], [s, c]], dtype=np.float64)


def _lift1(gate, wire):
    ops = [np.eye(2)] * N_QUBITS
    ops[wire] = gate
    out = ops[0]
    for o in ops[1:]:
        out = np.kron(out, o)
    return out


def _cnot(ctrl, tgt):
    U = np.zeros((16, 16))
    for i in range(16):
        bits = [(i >> (N_QUBITS - 1 - w)) & 1 for w in range(N_QUBITS)]
        if bits[ctrl] == 1:
            bits[tgt] ^= 1
        j = sum(b << (N_QUBITS - 1 - w) for w, b in enumerate(bits))
        U[j, i] = 1.0
    return U


def quad_form(q_params, post_w):
    """O (16x16 fp64): y = psi^T O psi + post_b."""
    qw = np.asarray(q_params, dtype=np.float64).reshape(Q_DEPTH, N_QUBITS)
    U = np.eye(16)
    for k in range(Q_DEPTH):
        U = _cnot(0, 1) @ U
        U = _cnot(2, 3) @ U
        U = _cnot(1, 2) @ U
        for w in range(N_QUBITS):
            U = _lift1(_ry(qw[k, w]), w) @ U
    Z = np.diag([1.0, -1.0])
    O = np.zeros((16, 16))
    pw = np.asarray(post_w, dtype=np.float64).reshape(-1)
    for w in range(N_QUBITS):
        O += pw[w] * (U.T @ _lift1(Z, w) @ U)
    return O


def _h_of_phi(phi):
    C, Sn = np.cos(2 * phi), np.sin(2 * phi)
    h = np.zeros((phi.shape[0], 16))
    for w in range(4):
        h[:, 2 * w] = C[:, w]
        h[:, 2 * w + 1] = Sn[:, w]
    h[:, 8] = h[:, 0] * h[:, 2]
    h[:, 9] = h[:, 0] * h[:, 3]
    h[:, 10] = h[:, 1] * h[:, 2]
    h[:, 11] = h[:, 1] * h[:, 3]
    h[:, 12] = h[:, 4] * h[:, 6]
    h[:, 13] = h[:, 4] * h[:, 7]
    h[:, 14] = h[:, 5] * h[:, 6]
    h[:, 15] = h[:, 5] * h[:, 7]
    return h


def solve_m16(O, post_b):
    """Symmetric 16x16 M with h^T M h = psi^T O psi + post_b for all angles."""
    rng = np.random.RandomState(12345)
    phi = rng.uniform(0, 2 * np.pi, (3000, 4))
    c, s = np.cos(phi), np.sin(phi)
    psi = np.einsum(
        'na,nb,nc,nd->nabcd',
        np.stack([c[:, 0], s[:, 0]], 1), np.stack([c[:, 1], s[:, 1]], 1),
        np.stack([c[:, 2], s[:, 2]], 1), np.stack([c[:, 3], s[:, 3]], 1),
    ).reshape(-1, 16)
    yv = np.einsum('ni,ij,nj->n', psi, O, psi) + post_b
    h = _h_of_phi(phi)
    A = np.einsum('ni,nj->nij', h, h).reshape(len(phi), 256)
    sol = np.linalg.lstsq(A, yv, rcond=None)[0]
    M = sol.reshape(16, 16)
    return 0.5 * (M + M.T)


def _consts(pre_w, pre_b, q_params, post_w, post_b):
    # wstack (128, 32) f16: [64j + f, 8i + j] = pre_w[i, f]; rest zero.
    wstack = np.zeros((128, 32), dtype=np.float32)
    for j in range(2):
        for i in range(4):
            wstack[64 * j:64 * j + 64, 8 * i + j] = pre_w[i, :]
    # bias (128, 1) f32: row r = 32v + 8i + j -> pre_b[i]
    biast = np.zeros((128, 1), dtype=np.float32)
    for r in range(128):
        biast[r, 0] = np.float32(pre_b[(r // 8) % 4])
    # H-block row decode: raw rows r = 32ph + 16half + 4v + 2wt + j,
    # product rows r = 64 + 8ps + 2v + j
    def _decode(r):
        if r < 64:
            ph, rr = divmod(r, 32)
            half, rr = divmod(rr, 16)
            v, rr = divmod(rr, 4)
            wt, j = divmod(rr, 2)
            return 2 * (2 * half + wt) + ph, 2 * v + j
        rr = r - 64
        ps, rr = divmod(rr, 8)
        v, j = divmod(rr, 2)
        return 8 + ps, 2 * v + j

    O = quad_form(q_params, post_w)
    M16 = solve_m16(O, post_b)
    dec = [_decode(r) for r in range(128)]
    m16bd = np.zeros((128, 128), dtype=np.float32)
    rsum4 = np.zeros((128, 128), dtype=np.float32)
    for r, (st, q) in enumerate(dec):
        for r2, (st2, q2) in enumerate(dec):
            if q == q2:
                m16bd[r, r2] = M16[st, st2]
        for k4 in range(4):
            rsum4[r, 32 * k4 + 8 * k4 + q] = 1.0
    blob = np.concatenate(
        [wstack.astype(np.float16), m16bd.astype(np.float16),
         rsum4.astype(np.float16)], axis=1)      # (128, 288)
    return np.ascontiguousarray(blob), biast


def _prep_x(x16):
    """(BATCH, 64) f16 -> (N_CORES, NM, 128, 4096) f16 in device layout."""
    v = x16.reshape(N_CORES, NM, 128, 8, 4, 2, 64)  # c, m, p, B, ch, j, f
    v = v.transpose(0, 1, 5, 6, 3, 4, 2)            # c, m, j, f, B, ch, p
    return np.ascontiguousarray(v).reshape(N_CORES, NM, 128, 4096)


def _out_perm():
    """index array: y_full[s] = y_dev.reshape(-1)[perm[s]] (per core)."""
    # y_dev[32m + 8*ut + q, 128*ch + p]; q = 2v+j
    # s = 8192 m + 64 p + 32 ut + 8 v + 2 ch + j
    idx = np.empty(S, dtype=np.int64)
    for m in range(NM):
        for ut in range(2):
            for v in range(4):
                for ch in range(4):
                    for j in range(2):
                        q = 2 * v + j
                        p = np.arange(128)
                        s = 8192 * m + 64 * p + 32 * ut + 8 * v + 2 * ch + j
                        idx[s] = (32 * m + 8 * ut + q) * 512 \
                            + 128 * ch + p
    return idx


# ---------------------------------------------------------------- program


def build(nm=NM):
    nc = bacc.Bacc()

    xt = nc.declare_dram_parameter("xt", (nm, 128, 4096), F16, isOutput=False)
    y = nc.declare_dram_parameter("y", (128, 512), F32, isOutput=True)
    blob_d = nc.declare_dram_parameter("blob", (128, 288), F16, isOutput=False)
    bias_d = nc.declare_dram_parameter("biast", (128, 1), F32, isOutput=False)

    PI2 = float(np.pi / 2)

    with tile.TileContext(nc) as tc:
        with (
            tc.tile_pool(name="const", bufs=1) as cpool,
            tc.tile_pool(name="xin", bufs=4) as xpool,
            tc.tile_pool(name="tbuf", bufs=3) as tpool,
            tc.tile_pool(name="ht", bufs=2) as htpool,
            tc.tile_pool(name="dp", bufs=3) as dpool,
            tc.tile_pool(name="yo", bufs=2) as ypool,
            tc.tile_pool(name="pers", bufs=1) as ppool,
            tc.tile_pool(name="psu", bufs=4, space="PSUM") as ps_u,
            tc.tile_pool(name="psm", bufs=2, space="PSUM") as ps_m,
            tc.tile_pool(name="psy", bufs=1, space="PSUM") as ps_y,
            tc.tile_pool(name="psw", bufs=1, space="PSUM") as ps_w,
        ):
            # constants: first on the sync HWDGE queue (wstack gates the
            # first matmul), biast on scalar
            blob = cpool.tile([128, 288], F16, tag="blob")
            biast = cpool.tile([128, 1], F32, tag="biast")
            nc.sync.dma_start(blob[:], blob_d[:])
            nc.scalar.dma_start(biast[:], bias_d[:])
            wstack = blob[:, 0:32]
            m16bd = blob[:, 32:160]
            rsum4 = blob[:, 160:288]
            b_zero = cpool.tile([128, 1], F32, tag="b_zero")
            b_pi2 = cpool.tile([128, 1], F32, tag="b_pi2")
            nc.gpsimd.memset(b_zero[:], 0.0)
            nc.gpsimd.memset(b_pi2[:], float(np.pi / 2))

            # PE p-state warmup: measured on HW — matmuls run 375-389ns
            # (2.4GHz) right after a dummy burst vs 605-634ns cold, and the
            # boost decays across ~3us idle gaps. Fill the startup window
            # and the x-wait / xbar-wait gaps with dummy matmuls.
            warm = cpool.tile([128, 512], F16, tag="warm")
            nc.gpsimd.memset(warm[:], 0.0)
            wout = ps_w.tile([128, 512], F32, tag="wout")

            def warmup(n):
                for _ in range(n):
                    nc.tensor.matmul(wout[:], warm[:, 0:128], warm[:])

            warmup(8)

            # persistent sample-major staging
            T2s = ppool.tile([128, 1024 * nm], F16, tag="t2s")
            H = ppool.tile([128, 1024 * nm], F16, tag="h")

            XTs = {}
            Ts = {}
            HTs = {}
            Yps = {}

            def load_x(m, eng, split=False):
                XT = xpool.tile([128, 4096], F16, tag="xt", name=f"XT{m}")
                if split:
                    # first B-slice alone so the first matmul starts ASAP
                    eng.dma_start(XT[:, 0:512], xt[m][:, 0:512])
                    eng.dma_start(XT[:, 512:2048], xt[m][:, 512:2048])
                    eng.dma_start(XT[:, 2048:4096], xt[m][:, 2048:4096])
                else:
                    eng.dma_start(XT[:], xt[m])
                XTs[m] = XT

            def front(m, xbar_eng=nc.sync):
                """pre-matmul + tanh + T-xbar for macro m."""
                XT = XTs.pop(m)
                T = tpool.tile([128, 1024], F16, tag="t", name=f"T{m}")
                for ut in range(2):
                    U = ps_u.tile([128, 512], F32, tag="u", name=f"U{m}_{ut}")
                    for v in range(4):
                        B = 4 * ut + v
                        nc.tensor.matmul(
                            U[32 * v:32 * v + 32, :], wstack,
                            XT[:, 512 * B:512 * B + 512],
                            tile_position=(0, 32 * v),
                        )
                    nc.scalar.activation(T[:, 512 * ut:512 * ut + 512], U[:],
                                         AF.Tanh, bias=biast[:])
                xbar_eng.dma_start(
                    T2s[:, 1024 * m:1024 * m + 1024].rearrange(
                        "q (c p) -> q c p", c=8, p=128),
                    T[:], transpose=True)
                Ts[m] = T

            def sincos(m):
                """raw double-angle values + products for macro m."""
                # T2s block: 32v + 8i + j; iterate (blk, v, i, j) so both
                # sides optimize to <=3D (in merges blk+v; out runs of 16)
                t_mac = T2s[:, 1024 * m:1024 * m + 1024].rearrange(
                    "p (blk v i j) -> p blk v i j", blk=8, v=4, i=4, j=8)
                h_raw = H[:, 1024 * m:1024 * m + 1024].rearrange(
                    "p (blk sec ph half v wt j) -> p sec ph half blk v wt j",
                    blk=8, sec=2, ph=2, half=2, v=4, wt=2, j=2)
                # cos pass (ph=0): Cw = -sin(pi/2 * t) ; sin pass: Sw = cos
                for half in range(2):
                    i0 = 2 * half
                    tin = t_mac[:, :, :, i0:i0 + 2, 0:2]
                    nc.scalar.activation(
                        h_raw[:, 0, 0, half], tin, AF.Sin,
                        bias=b_zero[:], scale=-PI2)
                    nc.scalar.activation(
                        h_raw[:, 0, 1, half], tin, AF.Sin,
                        bias=b_pi2[:], scale=PI2)

                def raw(ph, w):
                    half, wt = divmod(w, 2)
                    return h_raw[:, 0, ph, half, :, :, wt, :]

                h_prod = H[:, 1024 * m:1024 * m + 1024].rearrange(
                    "p (blk sec ps v j) -> p sec ps blk v j",
                    blk=8, sec=2, ps=8, v=4, j=2)
                for a in range(2):
                    for b in range(2):
                        nc.vector.tensor_tensor(
                            h_prod[:, 1, 2 * a + b], raw(a, 0), raw(b, 1),
                            ALU.mult)
                        nc.gpsimd.tensor_tensor(
                            h_prod[:, 1, 4 + 2 * a + b], raw(a, 2), raw(b, 3),
                            ALU.mult)

            def hxbar(m, eng):
                HT = htpool.tile([128, 1024], F16, tag="ht", name=f"HT{m}")
                eng.dma_start(
                    HT[:].rearrange("q (c p) -> q c p", c=8, p=128),
                    H[:, 1024 * m:1024 * m + 1024], transpose=True)
                HTs[m] = HT

            Yp = ps_y.tile([128, 512], F32, tag="yp", name="Yp")

            def quad(m):
                """MP + DP + RSUM for macro m (both halves)."""
                HT = HTs.pop(m)
                for ut in range(2):
                    rhs = HT[:, 512 * ut:512 * ut + 512]
                    MP = ps_m.tile([128, 512], F32, tag="mp",
                                   name=f"MP{m}_{ut}")
                    nc.tensor.matmul(MP[:], m16bd, rhs)
                    DP = dpool.tile([128, 512], F16, tag="dp",
                                    name=f"DP{m}_{ut}")
                    nc.vector.tensor_tensor(DP[:], rhs, MP[:], ALU.mult)
                    nc.tensor.matmul(
                        Yp[32 * m:32 * m + 32, :],
                        rsum4[:, 32 * ut:32 * ut + 32], DP[:],
                        tile_position=(0, 32 * m),
                        start=(ut == 0), stop=(ut == 1),
                        skip_group_check=True,
                    )

            def flush_all():
                Yo = ypool.tile([128, 512], F32, tag="yo", name="Yo")
                nc.scalar.activation(Yo[:], Yp[:], AF.Copy, bias=0.0)
                nc.sync.dma_start(y[:], Yo[:])

            # software-pipelined emission: back-half of macro m overlaps the
            # x-stream and fronts of macros m+1..
            # all x loads on the sync queue so DMA-pool service order
            # matches need order (cross-queue transfers serve issue-order)
            load_x(0, nc.sync, split=True)
            load_x(1, nc.sync)
            load_x(2, nc.sync)
            load_x(3, nc.sync)
            for m in range(nm):
                front(m)
                if m < nm - 1:
                    warmup(6)      # hold PE clock through the x-wait gap
            warmup(20)             # bridge the ~10us gap to the first MP
            for m in range(nm):
                sincos(m)
                hxbar(m, nc.sync)
                quad(m)
                if m < nm - 1:
                    warmup(3)      # hold PE clock through the xbar gap
            flush_all()

    return nc


# ---------------------------------------------------------------- entry


def kernel(input_features, pre_w, pre_b, q_params, post_w, post_b):
    global LAST_RESULTS
    x16 = np.asarray(input_features, np.float32).astype(np.float16)
    xt_all = _prep_x(x16)
    blob, biast = _consts(
        np.asarray(pre_w, np.float32), np.asarray(pre_b, np.float32),
        np.asarray(q_params, np.float32), np.asarray(post_w, np.float32),
        float(np.asarray(post_b).reshape(-1)[0]))

    nc = build(nm=NM)

    in_maps = [
        dict(xt=np.ascontiguousarray(xt_all[c]), blob=blob, biast=biast)
        for c in range(N_CORES)
    ]
    nc.finalize()
    res = run_bass_kernel_spmd(nc, in_maps, list(range(N_CORES)), trace=TRACE)
    LAST_RESULTS = res
    perm = _out_perm()
    outs = []
    for c in range(N_CORES):
        yd = np.asarray(res.results[c]["y"], np.float32).reshape(-1)
        outs.append(yd[perm])
    return np.concatenate(outs).reshape(BATCH, 1).astype(np.float32)


if __name__ == "__main__":
    print("kernel module OK")


# revision 62
# speedup vs baseline: 1.0127x; 1.0127x over previous
"""Trainium2 Bass kernel for nn_DressedQuantumNet (262144 x 64 -> 262144 x 1).

Design G: host-pretransposed input (pure linear DMAs) + double-angle
quadratic form.

Math: with t = tanh(u), u = x @ pre_w.T + pre_b, the circuit output is
    y = h^T M16 h,   h = (C0,S0,C1,S1,C2,S2,C3,S3, C0C1,C0S1,S0C1,S0S1,
                          C2C3,C2S3,S2C3,S2S3)
where Cw = cos 2phi_w = -sin((pi/2) t_w), Sw = sin 2phi_w = cos((pi/2) t_w),
and M16 (16x16 symmetric, absorbing post_w/post_b via c^2+s^2=1 identities)
is solved on host by least squares.

Per-core layout (S = 32768 samples, 4 macros of 8192):
  sample s = 8192 m + 64 p + 32 ut + 8 v + 2 c + j   (p<128, ut<2, v<4, c<4, j<2)
  xt dram [m][64j+f, 512(4ut+v) + 128c + p] = x[s, f]        (host-baked)
  pre-matmul K=128 (2 samples j), M=32 blocks (4v): U[32v+4j+i, 128c+p]
  tanh -> T f16 [128, 1024] per m;  xbar -> T2s[p, 128(4ut+c) + 32v+4j+i]
  ACT sin/cos -> H[p, 1024m + 128(4ut+c) + 8st + (2v+j)]  raw slots st=2w+ph
  DVE products -> slots 8..15;  xbar H -> HT[8st+q, 128(4ut+c)+p]
  MP = m16bd^T HT (q-interleaved blockdiag), DP = HT*MP,
  RSUM k4=2(m%2)+ut accumulates into Yp rows 8 k4 + q -> y[P, 32, 512] f32.
Host un-permutes the output.
"""
import sys

import numpy as np

for _p in ("/opt/trn_rl_repo",):
    if _p not in sys.path:
        sys.path.insert(0, _p)

import concourse.bass as bass
import concourse.bacc as bacc
import concourse.hw_specs as _hw_specs

_orig_get_act_tables = _hw_specs.get_activation_tables


def _pinned_act_tables(module_arch):
    tabs = _orig_get_act_tables(module_arch)
    if "silu_and_others" in tabs:
        tabs = {k: (v if k == "silu_and_others" else set())
                for k, v in tabs.items()}
    return tabs


bacc.get_activation_tables = _pinned_act_tables
import concourse.mybir as mybir
from concourse import tile
from concourse.bass_utils import run_bass_kernel_spmd

AF = mybir.ActivationFunctionType
ALU = mybir.AluOpType
F32 = mybir.dt.float32
F16 = mybir.dt.float16

N_CORES = 8
BATCH = 262144
S = BATCH // N_CORES          # 32768 samples per core
NM = 4                        # macros per core (8192 samples each)
N_QUBITS = 4
Q_DEPTH = 6
IN_F = 64

TRACE = False
LAST_RESULTS = None

# ---------------------------------------------------------------- host math


def _ry(theta):
    c, s = np.cos(theta / 2), np.sin(theta / 2)
    return np.array([[c, -s], [s, c]], dtype=np.float64)


def _lift1(gate, wire):
    ops = [np.eye(2)] * N_QUBITS
    ops[wire] = gate
    out = ops[0]
    for o in ops[1:]:
        out = np.kron(out, o)
    return out


def _cnot(ctrl, tgt):
    U = np.zeros((16, 16))
    for i in range(16):
        bits = [(i >> (N_QUBITS - 1 - w)) & 1 for w in range(N_QUBITS)]
        if bits[ctrl] == 1:
            bits[tgt] ^= 1
        j = sum(b << (N_QUBITS - 1 - w) for w, b in enumerate(bits))
        U[j, i] = 1.0
    return U


def quad_form(q_params, post_w):
    """O (16x16 fp64): y = psi^T O psi + post_b."""
    qw = np.asarray(q_params, dtype=np.float64).reshape(Q_DEPTH, N_QUBITS)
    U = np.eye(16)
    for k in range(Q_DEPTH):
        U = _cnot(0, 1) @ U
        U = _cnot(2, 3) @ U
        U = _cnot(1, 2) @ U
        for w in range(N_QUBITS):
            U = _lift1(_ry(qw[k, w]), w) @ U
    Z = np.diag([1.0, -1.0])
    O = np.zeros((16, 16))
    pw = np.asarray(post_w, dtype=np.float64).reshape(-1)
    for w in range(N_QUBITS):
        O += pw[w] * (U.T @ _lift1(Z, w) @ U)
    return O


def _h_of_phi(phi):
    C, Sn = np.cos(2 * phi), np.sin(2 * phi)
    h = np.zeros((phi.shape[0], 16))
    for w in range(4):
        h[:, 2 * w] = C[:, w]
        h[:, 2 * w + 1] = Sn[:, w]
    h[:, 8] = h[:, 0] * h[:, 2]
    h[:, 9] = h[:, 0] * h[:, 3]
    h[:, 10] = h[:, 1] * h[:, 2]
    h[:, 11] = h[:, 1] * h[:, 3]
    h[:, 12] = h[:, 4] * h[:, 6]
    h[:, 13] = h[:, 4] * h[:, 7]
    h[:, 14] = h[:, 5] * h[:, 6]
    h[:, 15] = h[:, 5] * h[:, 7]
    return h


def solve_m16(O, post_b):
    """Symmetric 16x16 M with h^T M h = psi^T O psi + post_b for all angles."""
    rng = np.random.RandomState(12345)
    phi = rng.uniform(0, 2 * np.pi, (3000, 4))
    c, s = np.cos(phi), np.sin(phi)
    psi = np.einsum(
        'na,nb,nc,nd->nabcd',
        np.stack([c[:, 0], s[:, 0]], 1), np.stack([c[:, 1], s[:, 1]], 1),
        np.stack([c[:, 2], s[:, 2]], 1), np.stack([c[:, 3], s[:, 3]], 1),
    ).reshape(-1, 16)
    yv = np.einsum('ni,ij,nj->n', psi, O, psi) + post_b
    h = _h_of_phi(phi)
    A = np.einsum('ni,nj->nij', h, h).reshape(len(phi), 256)
    sol = np.linalg.lstsq(A, yv, rcond=None)[0]
    M = sol.reshape(16, 16)
    return 0.5 * (M + M.T)


def _consts(pre_w, pre_b, q_params, post_w, post_b):
    # wstack (128, 32) f16: [64j + f, 8i + j] = pre_w[i, f]; rest zero.
    wstack = np.zeros((128, 32), dtype=np.float32)
    for j in range(2):
        for i in range(4):
            wstack[64 * j:64 * j + 64, 8 * i + j] = pre_w[i, :]
    # bias (128, 1) f32: row r = 32v + 8i + j -> pre_b[i]
    biast = np.zeros((128, 1), dtype=np.float32)
    for r in range(128):
        biast[r, 0] = np.float32(pre_b[(r // 8) % 4])
    # H-block row decode: raw rows r = 32ph + 16half + 4v + 2wt + j,
    # product rows r = 64 + 8ps + 2v + j
    def _decode(r):
        if r < 64:
            ph, rr = divmod(r, 32)
            half, rr = divmod(rr, 16)
            v, rr = divmod(rr, 4)
            wt, j = divmod(rr, 2)
            return 2 * (2 * half + wt) + ph, 2 * v + j
        rr = r - 64
        ps, rr = divmod(rr, 8)
        v, j = divmod(rr, 2)
        return 8 + ps, 2 * v + j

    O = quad_form(q_params, post_w)
    M16 = solve_m16(O, post_b)
    dec = [_decode(r) for r in range(128)]
    m16bd = np.zeros((128, 128), dtype=np.float32)
    rsum4 = np.zeros((128, 128), dtype=np.float32)
    for r, (st, q) in enumerate(dec):
        for r2, (st2, q2) in enumerate(dec):
            if q == q2:
                m16bd[r, r2] = M16[st, st2]
        for k4 in range(4):
            rsum4[r, 32 * k4 + 8 * k4 + q] = 1.0
    blob = np.concatenate(
        [wstack.astype(np.float16), m16bd.astype(np.float16),
         rsum4.astype(np.float16)], axis=1)      # (128, 288)
    return np.ascontiguousarray(blob), biast


def _prep_x(x16):
    """(BATCH, 64) f16 -> (N_CORES, NM, 128, 4096) f16 in device layout."""
    v = x16.reshape(N_CORES, NM, 128, 8, 4, 2, 64)  # c, m, p, B, ch, j, f
    v = v.transpose(0, 1, 5, 6, 3, 4, 2)            # c, m, j, f, B, ch, p
    return np.ascontiguousarray(v).reshape(N_CORES, NM, 128, 4096)


def _out_perm():
    """index array: y_full[s] = y_dev.reshape(-1)[perm[s]] (per core)."""
    # y_dev[32m + 8*ut + q, 128*ch + p]; q = 2v+j
    # s = 8192 m + 64 p + 32 ut + 8 v + 2 ch + j
    idx = np.empty(S, dtype=np.int64)
    for m in range(NM):
        for ut in range(2):
            for v in range(4):
                for ch in range(4):
                    for j in range(2):
                        q = 2 * v + j
                        p = np.arange(128)
                        s = 8192 * m + 64 * p + 32 * ut + 8 * v + 2 * ch + j
                        idx[s] = (32 * m + 8 * ut + q) * 512 \
                            + 128 * ch + p
    return idx


# ---------------------------------------------------------------- program


def build(nm=NM):
    nc = bacc.Bacc()

    xt = nc.declare_dram_parameter("xt", (nm, 128, 4096), F16, isOutput=False)
    y = nc.declare_dram_parameter("y", (128, 512), F32, isOutput=True)
    blob_d = nc.declare_dram_parameter("blob", (128, 288), F16, isOutput=False)
    bias_d = nc.declare_dram_parameter("biast", (128, 1), F32, isOutput=False)

    PI2 = float(np.pi / 2)

    with tile.TileContext(nc) as tc:
        with (
            tc.tile_pool(name="const", bufs=1) as cpool,
            tc.tile_pool(name="xin", bufs=4) as xpool,
            tc.tile_pool(name="tbuf", bufs=3) as tpool,
            tc.tile_pool(name="ht", bufs=2) as htpool,
            tc.tile_pool(name="dp", bufs=3) as dpool,
            tc.tile_pool(name="yo", bufs=2) as ypool,
            tc.tile_pool(name="pers", bufs=1) as ppool,
            tc.tile_pool(name="psu", bufs=4, space="PSUM") as ps_u,
            tc.tile_pool(name="psm", bufs=2, space="PSUM") as ps_m,
            tc.tile_pool(name="psy", bufs=1, space="PSUM") as ps_y,
            tc.tile_pool(name="psw", bufs=1, space="PSUM") as ps_w,
        ):
            # constants: first on the sync HWDGE queue (wstack gates the
            # first matmul), biast on scalar
            blob = cpool.tile([128, 288], F16, tag="blob")
            biast = cpool.tile([128, 1], F32, tag="biast")
            nc.sync.dma_start(blob[:], blob_d[:])
            nc.scalar.dma_start(biast[:], bias_d[:])
            wstack = blob[:, 0:32]
            m16bd = blob[:, 32:160]
            rsum4 = blob[:, 160:288]
            b_zero = cpool.tile([128, 1], F32, tag="b_zero")
            b_pi2 = cpool.tile([128, 1], F32, tag="b_pi2")
            nc.gpsimd.memset(b_zero[:], 0.0)
            nc.gpsimd.memset(b_pi2[:], float(np.pi / 2))

            # PE p-state warmup: measured on HW — matmuls run 375-389ns
            # (2.4GHz) right after a dummy burst vs 605-634ns cold, and the
            # boost decays across ~3us idle gaps. Fill the startup window
            # and the x-wait / xbar-wait gaps with dummy matmuls.
            warm = cpool.tile([128, 512], F16, tag="warm")
            nc.gpsimd.memset(warm[:], 0.0)
            wout = ps_w.tile([128, 512], F32, tag="wout")

            def warmup(n):
                for _ in range(n):
                    nc.tensor.matmul(wout[:], warm[:, 0:128], warm[:])

            warmup(8)

            # persistent sample-major staging
            T2s = ppool.tile([128, 1024 * nm], F16, tag="t2s")
            H = ppool.tile([128, 1024 * nm], F16, tag="h")

            XTs = {}
            Ts = {}
            HTs = {}
            Yps = {}

            def load_x(m, eng, split=False):
                XT = xpool.tile([128, 4096], F16, tag="xt", name=f"XT{m}")
                if split:
                    # first B-slice alone so the first matmul starts ASAP
                    eng.dma_start(XT[:, 0:512], xt[m][:, 0:512])
                    eng.dma_start(XT[:, 512:2048], xt[m][:, 512:2048])
                    eng.dma_start(XT[:, 2048:4096], xt[m][:, 2048:4096])
                else:
                    eng.dma_start(XT[:], xt[m])
                XTs[m] = XT

            def front(m, xbar_eng=nc.sync):
                """pre-matmul + tanh + T-xbar for macro m."""
                XT = XTs.pop(m)
                T = tpool.tile([128, 1024], F16, tag="t", name=f"T{m}")
                for ut in range(2):
                    U = ps_u.tile([128, 512], F32, tag="u", name=f"U{m}_{ut}")
                    for v in range(4):
                        B = 4 * ut + v
                        nc.tensor.matmul(
                            U[32 * v:32 * v + 32, :], wstack,
                            XT[:, 512 * B:512 * B + 512],
                            tile_position=(0, 32 * v),
                        )
                    nc.scalar.activation(T[:, 512 * ut:512 * ut + 512], U[:],
                                         AF.Tanh, bias=biast[:])
                xbar_eng.dma_start(
                    T2s[:, 1024 * m:1024 * m + 1024].rearrange(
                        "q (c p) -> q c p", c=8, p=128),
                    T[:], transpose=True)
                Ts[m] = T

            def sincos(m):
                """raw double-angle values + products for macro m."""
                # T2s block: 32v + 8i + j; iterate (blk, v, i, j) so both
                # sides optimize to <=3D (in merges blk+v; out runs of 16)
                t_mac = T2s[:, 1024 * m:1024 * m + 1024].rearrange(
                    "p (blk v i j) -> p blk v i j", blk=8, v=4, i=4, j=8)
                h_raw = H[:, 1024 * m:1024 * m + 1024].rearrange(
                    "p (blk sec ph half v wt j) -> p sec ph half blk v wt j",
                    blk=8, sec=2, ph=2, half=2, v=4, wt=2, j=2)
                # cos pass (ph=0): Cw = -sin(pi/2 * t) ; sin pass: Sw = cos
                for half in range(2):
                    i0 = 2 * half
                    tin = t_mac[:, :, :, i0:i0 + 2, 0:2]
                    nc.scalar.activation(
                        h_raw[:, 0, 0, half], tin, AF.Sin,
                        bias=b_zero[:], scale=-PI2)
                    nc.scalar.activation(
                        h_raw[:, 0, 1, half], tin, AF.Sin,
                        bias=b_pi2[:], scale=PI2)

                def raw(ph, w):
                    half, wt = divmod(w, 2)
                    return h_raw[:, 0, ph, half, :, :, wt, :]

                h_prod = H[:, 1024 * m:1024 * m + 1024].rearrange(
                    "p (blk sec ps v j) -> p sec ps blk v j",
                    blk=8, sec=2, ps=8, v=4, j=2)
                for a in range(2):
                    for b in range(2):
                        nc.vector.tensor_tensor(
                            h_prod[:, 1, 2 * a + b], raw(a, 0), raw(b, 1),
                            ALU.mult)
                        nc.gpsimd.tensor_tensor(
                            h_prod[:, 1, 4 + 2 * a + b], raw(a, 2), raw(b, 3),
                            ALU.mult)

            def hxbar(m, eng):
                HT = htpool.tile([128, 1024], F16, tag="ht", name=f"HT{m}")
                eng.dma_start(
                    HT[:].rearrange("q (c p) -> q c p", c=8, p=128),
                    H[:, 1024 * m:1024 * m + 1024], transpose=True)
                HTs[m] = HT

            Yp = ps_y.tile([128, 512], F32, tag="yp", name="Yp")

            def quad(m):
                """MP + DP + RSUM for macro m (both halves)."""
                HT = HTs.pop(m)
                for ut in range(2):
                    rhs = HT[:, 512 * ut:512 * ut + 512]
                    MP = ps_m.tile([128, 512], F32, tag="mp",
                                   name=f"MP{m}_{ut}")
                    nc.tensor.matmul(MP[:], m16bd, rhs)
                    DP = dpool.tile([128, 512], F16, tag="dp",
                                    name=f"DP{m}_{ut}")
                    nc.vector.tensor_tensor(DP[:], rhs, MP[:], ALU.mult)
                    nc.tensor.matmul(
                        Yp[32 * m:32 * m + 32, :],
                        rsum4[:, 32 * ut:32 * ut + 32], DP[:],
                        tile_position=(0, 32 * m),
                        start=(ut == 0), stop=(ut == 1),
                        skip_group_check=True,
                    )

            def flush_all():
                Yo = ypool.tile([128, 512], F32, tag="yo", name="Yo")
                nc.scalar.activation(Yo[:], Yp[:], AF.Copy, bias=0.0)
                nc.sync.dma_start(y[:], Yo[:])

            # software-pipelined emission: back-half of macro m overlaps the
            # x-stream and fronts of macros m+1..
            # all x loads on the sync queue so DMA-pool service order
            # matches need order (cross-queue transfers serve issue-order)
            load_x(0, nc.sync, split=True)
            load_x(1, nc.sync)
            load_x(2, nc.sync)
            load_x(3, nc.sync)
            for m in range(nm):
                front(m)
                if m < nm - 1:
                    # hold PE clock through the x-wait gap; the gap before
                    # the last macro measured ~2.2us (4 warm dummies only
                    # cover ~1.5us), so lengthen the final fill
                    warmup(4 if m < nm - 2 else 6)
            for m in range(nm):
                sincos(m)
                hxbar(m, nc.sync)
                quad(m)
                if m < nm - 1:
                    warmup(2)      # hold PE clock through the xbar gap
            flush_all()

    return nc


# ---------------------------------------------------------------- entry


def kernel(input_features, pre_w, pre_b, q_params, post_w, post_b):
    global LAST_RESULTS
    x16 = np.asarray(input_features, np.float32).astype(np.float16)
    xt_all = _prep_x(x16)
    blob, biast = _consts(
        np.asarray(pre_w, np.float32), np.asarray(pre_b, np.float32),
        np.asarray(q_params, np.float32), np.asarray(post_w, np.float32),
        float(np.asarray(post_b).reshape(-1)[0]))

    nc = build(nm=NM)

    in_maps = [
        dict(xt=np.ascontiguousarray(xt_all[c]), blob=blob, biast=biast)
        for c in range(N_CORES)
    ]
    nc.finalize()
    res = run_bass_kernel_spmd(nc, in_maps, list(range(N_CORES)), trace=TRACE)
    LAST_RESULTS = res
    perm = _out_perm()
    outs = []
    for c in range(N_CORES):
        yd = np.asarray(res.results[c]["y"], np.float32).reshape(-1)
        outs.append(yd[perm])
    return np.concatenate(outs).reshape(BATCH, 1).astype(np.float32)


if __name__ == "__main__":
    print("kernel module OK")
